# revision 6
# baseline (speedup 1.0000x reference)
"""MoE feed-forward (dense all-expert formulation) on 8 trn2 NeuronCores.

Expert-parallel: core e computes expert e's MLP over all tokens plus the
(replicated) router, scales by the renormalized top-2 routing weight, and a
ReduceScatter over the expert axis produces each core's slice of the summed
output.

Numerics: the two big matmuls run in fp32r (trn2's full-rate 20-bit fp32
mode: 1s/8e/11m). Weights are pre-rounded to fp32r on the host; activations
are rounded on-device at the PSUM-eviction copies. Products of fp32r values
are exact in fp32, so the only losses are the input roundings (~1.2e-4
relative) and fp32 accumulation. The router runs in plain fp32 because the
smallest top2/top3 logit margin decides expert selection and must match the
fp32 reference.
"""
import sys

sys.path.insert(0, "/opt/trn_rl_repo")

import numpy as np

import concourse.bass as bass
import concourse.mybir as mybir
import concourse.tile as tile
from concourse import bacc
from concourse.bass_utils import run_bass_kernel_spmd
from concourse.masks import make_identity

P = 128
B, S, D, H, E = 4, 2048, 1024, 4096, 8
NT = B * S                 # 8192 tokens
TB = 512                   # tokens per block
NTB = NT // TB             # 16
TT = TB // P               # 4 token subtiles per block
DT = D // P                # 8 d-tiles
HT = H // P                # 32 h-tiles
NCORES = 8

F32 = mybir.dt.float32
F32R = mybir.dt.float32r
AF = mybir.ActivationFunctionType
ALU = mybir.AluOpType


def round_fp32r(x: np.ndarray) -> np.ndarray:
    """Round fp32 to fp32r (1s+8e+11m; low 12 bits zero), round-to-nearest-even."""
    u = np.ascontiguousarray(x, np.float32).view(np.uint32)
    low = u & np.uint32(0xFFF)
    u = u & np.uint32(0xFFFFF000)
    half = np.uint32(0x800)
    lsb = (u >> np.uint32(12)) & np.uint32(1)
    round_up = (low > half) | ((low == half) & (lsb == 1))
    u = u + (round_up.astype(np.uint32) << np.uint32(12))
    return u.view(np.float32)


def build_kernel():
    nc = bacc.Bacc("TRN2", target_bir_lowering=False, debug=False,
                   num_devices=NCORES)

    x = nc.dram_tensor("x", [NT, D], F32, kind="ExternalInput")
    w1 = nc.dram_tensor("w1", [D, H], F32R, kind="ExternalInput")
    w2 = nc.dram_tensor("w2", [H, D], F32R, kind="ExternalInput")
    b1v = nc.dram_tensor("b1v", [H], F32, kind="ExternalInput")
    b2v = nc.dram_tensor("b2v", [D], F32, kind="ExternalInput")
    wr = nc.dram_tensor("wr", [D, E], F32, kind="ExternalInput")
    brv = nc.dram_tensor("brv", [E], F32, kind="ExternalInput")
    # one-hot selector of this core's expert column (program is shared by all
    # cores; only the inputs differ per core)
    esel = nc.dram_tensor("esel", [E, 1], F32, kind="ExternalInput")

    contrib = nc.dram_tensor("contrib", [D, NT], F32)                 # d-major
    rsout = nc.dram_tensor("rsout", [D // NCORES * NT], F32)
    y = nc.dram_tensor("y", [D // NCORES, NT], F32, kind="ExternalOutput")

    with tile.TileContext(nc) as tc:
        with tc.tile_pool(name="const", bufs=1) as cst, \
             tc.tile_pool(name="xin", bufs=4) as xin_p, \
             tc.tile_pool(name="xt32", bufs=10) as xt32_p, \
             tc.tile_pool(name="xtr", bufs=10) as xtr_p, \
             tc.tile_pool(name="ht", bufs=HT + 1) as ht_p, \
             tc.tile_pool(name="w1p", bufs=3) as w1_p, \
             tc.tile_pool(name="w2p", bufs=2) as w2_p, \
             tc.tile_pool(name="outp", bufs=3) as out_p, \
             tc.tile_pool(name="rt", bufs=3) as rt_p, \
             tc.tile_pool(name="ps1", bufs=2, space="PSUM") as ps1_p, \
             tc.tile_pool(name="ps2", bufs=2, space="PSUM") as ps2_p, \
             tc.tile_pool(name="psm", bufs=3, space="PSUM") as psm_p:

            # ---- constants ----
            ident = cst.tile([P, P], F32)
            make_identity(nc, ident[:])
            ones1 = cst.tile([1, P], F32)
            nc.vector.memset(ones1[:], 1.0)
            b1_sb = cst.tile([P, HT], F32)
            nc.sync.dma_start(out=b1_sb[:], in_=b1v[:].rearrange("(h p) -> p h", p=P))
            b2_sb = cst.tile([P, DT], F32)
            nc.sync.dma_start(out=b2_sb[:], in_=b2v[:].rearrange("(d p) -> p d", p=P))
            wr_sb = cst.tile([P, DT * E], F32)
            nc.sync.dma_start(out=wr_sb[:].rearrange("p (k e) -> p k e", k=DT),
                              in_=wr[:].rearrange("(k p) e -> p k e", p=P))
            br_sb = cst.tile([E, 1], F32)
            nc.sync.dma_start(out=br_sb[:], in_=brv[:].rearrange("(e o) -> e o", o=1))
            esel_sb = cst.tile([E, 1], F32)
            nc.sync.dma_start(out=esel_sb[:], in_=esel[:])

            for tb in range(NTB):
                t0 = tb * TB
                # ---- load x block and transpose to d-major ----
                xin = []
                for tt in range(TT):
                    xi = xin_p.tile([P, D], F32, tag="xin")
                    nc.sync.dma_start(out=xi[:], in_=x[t0 + tt * P: t0 + (tt + 1) * P, :])
                    xin.append(xi)
                xt32 = []
                xtr = []
                for dt in range(DT):
                    x32 = xt32_p.tile([P, TB], F32, tag="xt32")
                    for tt in range(TT):
                        pt = psm_p.tile([P, P], F32, space="PSUM", tag="psm")
                        nc.tensor.transpose(pt[:], xin[tt][:, dt * P:(dt + 1) * P], ident[:])
                        nc.scalar.activation(x32[:, tt * P:(tt + 1) * P], pt[:], AF.Copy)
                    xr = xtr_p.tile([P, TB], F32R, tag="xtr")
                    nc.vector.tensor_copy(xr[:], x32[:])
                    xt32.append(x32)
                    xtr.append(xr)

                # ---- router: logitsT [E, TB] in fp32 ----
                lg_ps = psm_p.tile([E, TB], F32, space="PSUM", tag="psm")
                for k in range(DT):
                    nc.tensor.matmul(out=lg_ps[:],
                                     lhsT=wr_sb[:].rearrange("p (k e) -> p k e", k=DT)[:, k, :],
                                     rhs=xt32[k][:],
                                     start=(k == 0), stop=(k == DT - 1))
                lgT = rt_p.tile([E, TB], F32, tag="lgT")
                nc.vector.tensor_scalar_add(lgT[:], lg_ps[:], br_sb[:, :1])
                # transpose to token-major [P, TT*E]
                lg_tok = rt_p.tile([P, TT * E], F32, tag="lgtok")
                for tt in range(TT):
                    pt = psm_p.tile([P, E], F32, space="PSUM", tag="psm")
                    nc.tensor.matmul(out=pt[:], lhsT=lgT[:, tt * P:(tt + 1) * P],
                                     rhs=ident[:E, :E], is_transpose=True,
                                     start=True, stop=True)
                    nc.scalar.activation(lg_tok[:, tt * E:(tt + 1) * E], pt[:], AF.Copy)

                v = lg_tok[:].rearrange("p (t e) -> p t e", e=E)
                m1 = rt_p.tile([P, TT], F32, tag="m1")
                nc.vector.tensor_reduce(m1[:], v, axis=mybir.AxisListType.X, op=ALU.max)
                eq = rt_p.tile([P, TT * E], F32, tag="eq")
                eqv = eq[:].rearrange("p (t e) -> p t e", e=E)
                nc.vector.tensor_tensor(out=eqv, in0=v,
                                        in1=m1[:].unsqueeze(2).to_broadcast([P, TT, E]),
                                        op=ALU.is_equal)
                tmp = rt_p.tile([P, TT * E], F32, tag="tmp")
                nc.vector.tensor_scalar(out=tmp[:], in0=eq[:], scalar1=-1.0e30,
                                        scalar2=None, op0=ALU.mult)
                nc.vector.tensor_tensor(out=tmp[:], in0=tmp[:], in1=lg_tok[:], op=ALU.add)
                m2 = rt_p.tile([P, TT], F32, tag="m2")
                nc.vector.tensor_reduce(m2[:], tmp[:].rearrange("p (t e) -> p t e", e=E),
                                        axis=mybir.AxisListType.X, op=ALU.max)
                m1n = rt_p.tile([P, TT], F32, tag="m1n")
                nc.vector.tensor_scalar(out=m1n[:], in0=m1[:], scalar1=-1.0,
                                        scalar2=None, op0=ALU.mult)
                d2 = rt_p.tile([P, TT], F32, tag="d2")
                nc.vector.tensor_tensor(out=d2[:], in0=m2[:], in1=m1n[:], op=ALU.add)
                e2 = rt_p.tile([P, TT], F32, tag="e2")
                nc.scalar.activation(e2[:], d2[:], AF.Exp)
                den = rt_p.tile([P, TT], F32, tag="den")
                nc.vector.tensor_scalar(out=den[:], in0=e2[:], scalar1=1.0,
                                        scalar2=None, op0=ALU.add)
                rden = rt_p.tile([P, TT], F32, tag="rden")
                nc.vector.reciprocal(rden[:], den[:])
                # le[p, t]: this core's expert logit, token-major. The program is
                # shared by all cores, so the expert column is selected with the
                # per-core one-hot input: le_row = esel.T @ lgT -> [1, TB], then a
                # per-subtile PE transpose gives the token-major [P, TT] layout.
                le_ps = psm_p.tile([1, TB], F32, space="PSUM", tag="psm")
                nc.tensor.matmul(out=le_ps[:], lhsT=esel_sb[:], rhs=lgT[:],
                                 start=True, stop=True)
                le_row = rt_p.tile([1, TB], F32, tag="lerow")
                nc.scalar.activation(le_row[:], le_ps[:], AF.Copy)
                le_tok = rt_p.tile([P, TT], F32, tag="letok")
                for tt in range(TT):
                    pt = psm_p.tile([P, 1], F32, space="PSUM", tag="psm")
                    nc.tensor.matmul(out=pt[:], lhsT=le_row[:, tt * P:(tt + 1) * P],
                                     rhs=ident[:1, :1], is_transpose=True,
                                     start=True, stop=True)
                    nc.scalar.activation(le_tok[:, tt:tt + 1], pt[:], AF.Copy)
                ge = rt_p.tile([P, TT], F32, tag="ge")
                nc.vector.tensor_tensor(out=ge[:], in0=le_tok[:], in1=m2[:], op=ALU.is_ge)
                d1 = rt_p.tile([P, TT], F32, tag="d1")
                nc.vector.tensor_tensor(out=d1[:], in0=le_tok[:], in1=m1n[:], op=ALU.add)
                p1 = rt_p.tile([P, TT], F32, tag="p1")
                nc.scalar.activation(p1[:], d1[:], AF.Exp)
                rw = rt_p.tile([P, TT], F32, tag="rw")
                nc.vector.tensor_tensor(out=rw[:], in0=p1[:], in1=rden[:], op=ALU.mult)
                nc.vector.tensor_tensor(out=rw[:], in0=rw[:], in1=ge[:], op=ALU.mult)

                # rw [P, TT] token-major -> rw_bcast [P, TB] (value per token column)
                rwb = rt_p.tile([P, TB], F32, tag="rwb")
                for tt in range(TT):
                    ptT = psm_p.tile([1, P], F32, space="PSUM", tag="psm")
                    nc.tensor.matmul(out=ptT[:], lhsT=rw[:, tt:tt + 1], rhs=ident[:],
                                     is_transpose=True, start=True, stop=True)
                    rwT_t = rt_p.tile([1, P], F32, tag="rwTt")
                    nc.scalar.activation(rwT_t[:], ptT[:], AF.Copy)
                    pb = psm_p.tile([P, P], F32, space="PSUM", tag="psm")
                    nc.tensor.matmul(out=pb[:], lhsT=ones1[:],
                                     rhs=rwT_t[:], start=True, stop=True)
                    nc.scalar.activation(rwb[:, tt * P:(tt + 1) * P], pb[:], AF.Copy)

                # ---- stage 1: hT[h, tok] = relu(W1.T-contract(xT)) + b1, fp32r ----
                ht_tiles = []
                for ht in range(HT):
                    w1t = w1_p.tile([P, DT * P], F32R, tag="w1t")
                    nc.sync.dma_start(
                        out=w1t[:].rearrange("p (k h) -> p k h", k=DT),
                        in_=w1[:, ht * P:(ht + 1) * P].rearrange("(k p) h -> p k h", p=P))
                    ps = ps1_p.tile([P, TB], F32, space="PSUM", tag="ps1")
                    w1v = w1t[:].rearrange("p (k h) -> p k h", k=DT)
                    for k in range(DT):
                        nc.tensor.matmul(out=ps[:], lhsT=w1v[:, k, :], rhs=xtr[k][:],
                                         start=(k == 0), stop=(k == DT - 1))
                    hti = ht_p.tile([P, TB], F32R, tag="ht")
                    nc.scalar.activation(hti[:], ps[:], AF.Relu,
                                         bias=b1_sb[:, ht:ht + 1])
                    ht_tiles.append(hti)

                # ---- stage 2: outT[d, tok] = W2.T-contract(hT) + b2, * rw ----
                for dt in range(DT):
                    w2t = w2_p.tile([P, HT * P], F32R, tag="w2t")
                    nc.sync.dma_start(
                        out=w2t[:].rearrange("p (k d) -> p k d", k=HT),
                        in_=w2[:, dt * P:(dt + 1) * P].rearrange("(k p) d -> p k d", p=P))
                    ps = ps2_p.tile([P, TB], F32, space="PSUM", tag="ps2")
                    w2v = w2t[:].rearrange("p (k d) -> p k d", k=HT)
                    for hk in range(HT):
                        nc.tensor.matmul(out=ps[:], lhsT=w2v[:, hk, :],
                                         rhs=ht_tiles[hk][:],
                                         start=(hk == 0), stop=(hk == HT - 1))
                    ot = out_p.tile([P, TB], F32, tag="ot")
                    nc.vector.tensor_scalar_add(ot[:], ps[:], b2_sb[:, dt:dt + 1])
                    ot2 = out_p.tile([P, TB], F32, tag="ot2")
                    nc.vector.tensor_tensor(out=ot2[:], in0=ot[:], in1=rwb[:], op=ALU.mult)
                    nc.sync.dma_start(
                        out=contrib[dt * P:(dt + 1) * P, t0:t0 + TB], in_=ot2[:])

            # ---- combine over experts: ReduceScatter, then copy out ----
            nc.gpsimd.collective_compute(
                "ReduceScatter", ALU.add,
                replica_groups=[list(range(NCORES))],
                ins=[contrib[:].opt()], outs=[rsout[:].opt()])
            nc.sync.dma_start(out=y[:], in_=rsout[:].rearrange("(p n) -> p n", p=P))

    nc.compile()
    return nc


_NC = None


def kernel(input_emb, W1, b1, W2, b2, Wr, br):
    global _NC
    if _NC is None:
        _NC = build_kernel()

    x = np.ascontiguousarray(np.asarray(input_emb, np.float32).reshape(NT, D))
    Wr_ = np.ascontiguousarray(np.asarray(Wr, np.float32))
    br_ = np.ascontiguousarray(np.asarray(br, np.float32))
    in_maps = []
    for e in range(NCORES):
        onehot = np.zeros((E, 1), np.float32)
        onehot[e, 0] = 1.0
        in_maps.append({
            "x": x,
            "w1": round_fp32r(np.asarray(W1[e], np.float32)),
            "w2": round_fp32r(np.asarray(W2[e], np.float32)),
            "b1v": np.ascontiguousarray(np.asarray(b1[e], np.float32)),
            "b2v": np.ascontiguousarray(np.asarray(b2[e], np.float32)),
            "wr": Wr_,
            "brv": br_,
            "esel": onehot,
        })
    r = run_bass_kernel_spmd(_NC, in_maps, core_ids=list(range(NCORES)))
    outT = np.concatenate([r.results[i]["y"] for i in range(NCORES)], axis=0)
    return np.ascontiguousarray(outT.T).reshape(B, S, D)


# revision 10
# speedup vs baseline: 1.0855x; 1.0855x over previous
"""MoE feed-forward (dense all-expert formulation) on 8 trn2 NeuronCores.

Expert-parallel: core e computes expert e's MLP over all tokens plus the
(replicated) router, scales by the renormalized top-2 routing weight, and a
ReduceScatter over the expert axis produces each core's slice of the summed
output.

Numerics: the two big matmuls run in fp32r (trn2's full-rate 20-bit fp32
mode: 1s/8e/11m). Weights are pre-rounded to fp32r on the host; activations
are rounded on-device at the PSUM-eviction copies. Products of fp32r values
are exact in fp32, so the only losses are the input roundings (~1.2e-4
relative) and fp32 accumulation. The router runs in plain fp32 because the
smallest top2/top3 logit margin decides expert selection and must match the
fp32 reference.
"""
import sys

sys.path.insert(0, "/opt/trn_rl_repo")

import numpy as np

import concourse.bass as bass
import concourse.mybir as mybir
import concourse.tile as tile
from concourse import bacc
from concourse.bass_utils import run_bass_kernel_spmd
from concourse.masks import make_identity

P = 128
B, S, D, H, E = 4, 2048, 1024, 4096, 8
NT = B * S                 # 8192 tokens
TB = 512                   # tokens per block
NTB = NT // TB             # 16
TT = TB // P               # 4 token subtiles per block
DT = D // P                # 8 d-tiles
HT = H // P                # 32 h-tiles
NCORES = 8

F32 = mybir.dt.float32
F32R = mybir.dt.float32r
AF = mybir.ActivationFunctionType
ALU = mybir.AluOpType


def round_fp32r(x: np.ndarray) -> np.ndarray:
    """Round fp32 to fp32r (1s+8e+11m; low 12 bits zero), round-to-nearest-even."""
    u = np.ascontiguousarray(x, np.float32).view(np.uint32)
    low = u & np.uint32(0xFFF)
    u = u & np.uint32(0xFFFFF000)
    half = np.uint32(0x800)
    lsb = (u >> np.uint32(12)) & np.uint32(1)
    round_up = (low > half) | ((low == half) & (lsb == 1))
    u = u + (round_up.astype(np.uint32) << np.uint32(12))
    return u.view(np.float32)


def build_kernel():
    nc = bacc.Bacc("TRN2", target_bir_lowering=False, debug=False,
                   num_devices=NCORES)

    x = nc.dram_tensor("x", [NT, D], F32, kind="ExternalInput")
    # Weights come in host-pre-tiled layouts so the streaming DMAs read
    # contiguous 4-16KB runs per partition row:
    #   w1[ht*128 + p, k*128 + h] = W1[k*128 + p, ht*128 + h]
    #   w2[dt*128 + p, hk*128 + d] = W2[hk*128 + p, dt*128 + d]
    w1 = nc.dram_tensor("w1", [H, D], F32R, kind="ExternalInput")
    w2 = nc.dram_tensor("w2", [D, H], F32R, kind="ExternalInput")
    b1v = nc.dram_tensor("b1v", [H], F32, kind="ExternalInput")
    b2v = nc.dram_tensor("b2v", [D], F32, kind="ExternalInput")
    wr = nc.dram_tensor("wr", [D, E], F32, kind="ExternalInput")
    brv = nc.dram_tensor("brv", [E], F32, kind="ExternalInput")
    # one-hot selector of this core's expert column (program is shared by all
    # cores; only the inputs differ per core)
    esel = nc.dram_tensor("esel", [E, 1], F32, kind="ExternalInput")

    contrib = nc.dram_tensor("contrib", [D, NT], F32)                 # d-major
    rsout = nc.dram_tensor("rsout", [D // NCORES * NT], F32)
    y = nc.dram_tensor("y", [D // NCORES, NT], F32, kind="ExternalOutput")

    with tile.TileContext(nc) as tc:
        with tc.tile_pool(name="const", bufs=1) as cst, \
             tc.tile_pool(name="xin", bufs=4) as xin_p, \
             tc.tile_pool(name="xt32", bufs=10) as xt32_p, \
             tc.tile_pool(name="xtr", bufs=10) as xtr_p, \
             tc.tile_pool(name="ht", bufs=HT + 1) as ht_p, \
             tc.tile_pool(name="w1p", bufs=3) as w1_p, \
             tc.tile_pool(name="w2p", bufs=2) as w2_p, \
             tc.tile_pool(name="outp", bufs=3) as out_p, \
             tc.tile_pool(name="rt", bufs=3) as rt_p, \
             tc.tile_pool(name="ps1", bufs=2, space="PSUM") as ps1_p, \
             tc.tile_pool(name="ps2", bufs=2, space="PSUM") as ps2_p, \
             tc.tile_pool(name="psm", bufs=3, space="PSUM") as psm_p:

            # ---- constants ----
            ident = cst.tile([P, P], F32)
            make_identity(nc, ident[:])
            ones1 = cst.tile([1, P], F32)
            nc.vector.memset(ones1[:], 1.0)
            b1_sb = cst.tile([P, HT], F32)
            nc.sync.dma_start(out=b1_sb[:], in_=b1v[:].rearrange("(h p) -> p h", p=P))
            b2_sb = cst.tile([P, DT], F32)
            nc.sync.dma_start(out=b2_sb[:], in_=b2v[:].rearrange("(d p) -> p d", p=P))
            wr_sb = cst.tile([P, DT * E], F32)
            nc.sync.dma_start(out=wr_sb[:].rearrange("p (k e) -> p k e", k=DT),
                              in_=wr[:].rearrange("(k p) e -> p k e", p=P))
            br_sb = cst.tile([E, 1], F32)
            nc.sync.dma_start(out=br_sb[:], in_=brv[:].rearrange("(e o) -> e o", o=1))
            esel_sb = cst.tile([E, 1], F32)
            nc.sync.dma_start(out=esel_sb[:], in_=esel[:])

            for tb in range(NTB):
                t0 = tb * TB
                # ---- load x block and transpose to d-major ----
                xin = []
                for tt in range(TT):
                    xi = xin_p.tile([P, D], F32, tag="xin")
                    nc.sync.dma_start(out=xi[:], in_=x[t0 + tt * P: t0 + (tt + 1) * P, :])
                    xin.append(xi)
                xt32 = []
                xtr = []
                for dt in range(DT):
                    x32 = xt32_p.tile([P, TB], F32, tag="xt32")
                    for tt in range(TT):
                        pt = psm_p.tile([P, P], F32, space="PSUM", tag="psm")
                        nc.tensor.transpose(pt[:], xin[tt][:, dt * P:(dt + 1) * P], ident[:])
                        nc.scalar.activation(x32[:, tt * P:(tt + 1) * P], pt[:], AF.Copy)
                    xr = xtr_p.tile([P, TB], F32R, tag="xtr")
                    nc.vector.tensor_copy(xr[:], x32[:])
                    xt32.append(x32)
                    xtr.append(xr)

                # ---- router: logitsT [E, TB] in fp32 ----
                lg_ps = psm_p.tile([E, TB], F32, space="PSUM", tag="psm")
                for k in range(DT):
                    nc.tensor.matmul(out=lg_ps[:],
                                     lhsT=wr_sb[:].rearrange("p (k e) -> p k e", k=DT)[:, k, :],
                                     rhs=xt32[k][:],
                                     start=(k == 0), stop=(k == DT - 1))
                lgT = rt_p.tile([E, TB], F32, tag="lgT")
                nc.vector.tensor_scalar_add(lgT[:], lg_ps[:], br_sb[:, :1])
                # transpose to token-major [P, TT*E]
                lg_tok = rt_p.tile([P, TT * E], F32, tag="lgtok")
                for tt in range(TT):
                    pt = psm_p.tile([P, E], F32, space="PSUM", tag="psm")
                    nc.tensor.matmul(out=pt[:], lhsT=lgT[:, tt * P:(tt + 1) * P],
                                     rhs=ident[:E, :E], is_transpose=True,
                                     start=True, stop=True)
                    nc.scalar.activation(lg_tok[:, tt * E:(tt + 1) * E], pt[:], AF.Copy)

                v = lg_tok[:].rearrange("p (t e) -> p t e", e=E)
                m1 = rt_p.tile([P, TT], F32, tag="m1")
                nc.vector.tensor_reduce(m1[:], v, axis=mybir.AxisListType.X, op=ALU.max)
                eq = rt_p.tile([P, TT * E], F32, tag="eq")
                eqv = eq[:].rearrange("p (t e) -> p t e", e=E)
                nc.vector.tensor_tensor(out=eqv, in0=v,
                                        in1=m1[:].unsqueeze(2).to_broadcast([P, TT, E]),
                                        op=ALU.is_equal)
                tmp = rt_p.tile([P, TT * E], F32, tag="tmp")
                nc.vector.tensor_scalar(out=tmp[:], in0=eq[:], scalar1=-1.0e30,
                                        scalar2=None, op0=ALU.mult)
                nc.vector.tensor_tensor(out=tmp[:], in0=tmp[:], in1=lg_tok[:], op=ALU.add)
                m2 = rt_p.tile([P, TT], F32, tag="m2")
                nc.vector.tensor_reduce(m2[:], tmp[:].rearrange("p (t e) -> p t e", e=E),
                                        axis=mybir.AxisListType.X, op=ALU.max)
                m1n = rt_p.tile([P, TT], F32, tag="m1n")
                nc.vector.tensor_scalar(out=m1n[:], in0=m1[:], scalar1=-1.0,
                                        scalar2=None, op0=ALU.mult)
                d2 = rt_p.tile([P, TT], F32, tag="d2")
                nc.vector.tensor_tensor(out=d2[:], in0=m2[:], in1=m1n[:], op=ALU.add)
                e2 = rt_p.tile([P, TT], F32, tag="e2")
                nc.scalar.activation(e2[:], d2[:], AF.Exp)
                den = rt_p.tile([P, TT], F32, tag="den")
                nc.vector.tensor_scalar(out=den[:], in0=e2[:], scalar1=1.0,
                                        scalar2=None, op0=ALU.add)
                rden = rt_p.tile([P, TT], F32, tag="rden")
                nc.vector.reciprocal(rden[:], den[:])
                # le[p, t]: this core's expert logit, token-major. The program is
                # shared by all cores, so the expert column is selected with the
                # per-core one-hot input: le_row = esel.T @ lgT -> [1, TB], then a
                # per-subtile PE transpose gives the token-major [P, TT] layout.
                le_ps = psm_p.tile([1, TB], F32, space="PSUM", tag="psm")
                nc.tensor.matmul(out=le_ps[:], lhsT=esel_sb[:], rhs=lgT[:],
                                 start=True, stop=True)
                le_row = rt_p.tile([1, TB], F32, tag="lerow")
                nc.scalar.activation(le_row[:], le_ps[:], AF.Copy)
                le_tok = rt_p.tile([P, TT], F32, tag="letok")
                for tt in range(TT):
                    pt = psm_p.tile([P, 1], F32, space="PSUM", tag="psm")
                    nc.tensor.matmul(out=pt[:], lhsT=le_row[:, tt * P:(tt + 1) * P],
                                     rhs=ident[:1, :1], is_transpose=True,
                                     start=True, stop=True)
                    nc.scalar.activation(le_tok[:, tt:tt + 1], pt[:], AF.Copy)
                ge = rt_p.tile([P, TT], F32, tag="ge")
                nc.vector.tensor_tensor(out=ge[:], in0=le_tok[:], in1=m2[:], op=ALU.is_ge)
                d1 = rt_p.tile([P, TT], F32, tag="d1")
                nc.vector.tensor_tensor(out=d1[:], in0=le_tok[:], in1=m1n[:], op=ALU.add)
                p1 = rt_p.tile([P, TT], F32, tag="p1")
                nc.scalar.activation(p1[:], d1[:], AF.Exp)
                rw = rt_p.tile([P, TT], F32, tag="rw")
                nc.vector.tensor_tensor(out=rw[:], in0=p1[:], in1=rden[:], op=ALU.mult)
                nc.vector.tensor_tensor(out=rw[:], in0=rw[:], in1=ge[:], op=ALU.mult)

                # rw [P, TT] token-major -> rw_bcast [P, TB] (value per token column)
                rwb = rt_p.tile([P, TB], F32, tag="rwb")
                for tt in range(TT):
                    ptT = psm_p.tile([1, P], F32, space="PSUM", tag="psm")
                    nc.tensor.matmul(out=ptT[:], lhsT=rw[:, tt:tt + 1], rhs=ident[:],
                                     is_transpose=True, start=True, stop=True)
                    rwT_t = rt_p.tile([1, P], F32, tag="rwTt")
                    nc.scalar.activation(rwT_t[:], ptT[:], AF.Copy)
                    pb = psm_p.tile([P, P], F32, space="PSUM", tag="psm")
                    nc.tensor.matmul(out=pb[:], lhsT=ones1[:],
                                     rhs=rwT_t[:], start=True, stop=True)
                    nc.scalar.activation(rwb[:, tt * P:(tt + 1) * P], pb[:], AF.Copy)

                # ---- stage 1: hT[h, tok] = relu(W1.T-contract(xT)) + b1, fp32r ----
                ht_tiles = []
                for ht in range(HT):
                    w1t = w1_p.tile([P, DT * P], F32R, tag="w1t")
                    nc.sync.dma_start(out=w1t[:], in_=w1[ht * P:(ht + 1) * P, :])
                    ps = ps1_p.tile([P, TB], F32, space="PSUM", tag="ps1")
                    w1v = w1t[:].rearrange("p (k h) -> p k h", k=DT)
                    for k in range(DT):
                        nc.tensor.matmul(out=ps[:], lhsT=w1v[:, k, :], rhs=xtr[k][:],
                                         start=(k == 0), stop=(k == DT - 1))
                    hti = ht_p.tile([P, TB], F32R, tag="ht")
                    nc.scalar.activation(hti[:], ps[:], AF.Relu,
                                         bias=b1_sb[:, ht:ht + 1])
                    ht_tiles.append(hti)

                # ---- stage 2: outT[d, tok] = W2.T-contract(hT) + b2, * rw ----
                for dt in range(DT):
                    w2t = w2_p.tile([P, HT * P], F32R, tag="w2t")
                    nc.sync.dma_start(out=w2t[:], in_=w2[dt * P:(dt + 1) * P, :])
                    ps = ps2_p.tile([P, TB], F32, space="PSUM", tag="ps2")
                    w2v = w2t[:].rearrange("p (k d) -> p k d", k=HT)
                    for hk in range(HT):
                        nc.tensor.matmul(out=ps[:], lhsT=w2v[:, hk, :],
                                         rhs=ht_tiles[hk][:],
                                         start=(hk == 0), stop=(hk == HT - 1))
                    ot = out_p.tile([P, TB], F32, tag="ot")
                    nc.vector.tensor_scalar_add(ot[:], ps[:], b2_sb[:, dt:dt + 1])
                    ot2 = out_p.tile([P, TB], F32, tag="ot2")
                    nc.vector.tensor_tensor(out=ot2[:], in0=ot[:], in1=rwb[:], op=ALU.mult)
                    nc.sync.dma_start(
                        out=contrib[dt * P:(dt + 1) * P, t0:t0 + TB], in_=ot2[:])

            # ---- combine over experts: ReduceScatter, then copy out ----
            nc.gpsimd.collective_compute(
                "ReduceScatter", ALU.add,
                replica_groups=[list(range(NCORES))],
                ins=[contrib[:].opt()], outs=[rsout[:].opt()])
            nc.sync.dma_start(out=y[:], in_=rsout[:].rearrange("(p n) -> p n", p=P))

    nc.compile()
    return nc


_NC = None


def tile_w1(W1e: np.ndarray) -> np.ndarray:
    """[D, H] -> [H, D] with w1[ht*128+p, k*128+h] = W1[k*128+p, ht*128+h]."""
    v = np.asarray(W1e, np.float32).reshape(DT, P, HT, P)
    return np.ascontiguousarray(v.transpose(2, 1, 0, 3).reshape(H, D))


def tile_w2(W2e: np.ndarray) -> np.ndarray:
    """[H, D] -> [D, H] with w2[dt*128+p, hk*128+d] = W2[hk*128+p, dt*128+d]."""
    v = np.asarray(W2e, np.float32).reshape(HT, P, DT, P)
    return np.ascontiguousarray(v.transpose(2, 1, 0, 3).reshape(D, H))


def make_in_maps(input_emb, W1, b1, W2, b2, Wr, br):
    x = np.ascontiguousarray(np.asarray(input_emb, np.float32).reshape(NT, D))
    Wr_ = np.ascontiguousarray(np.asarray(Wr, np.float32))
    br_ = np.ascontiguousarray(np.asarray(br, np.float32))
    in_maps = []
    for e in range(NCORES):
        onehot = np.zeros((E, 1), np.float32)
        onehot[e, 0] = 1.0
        in_maps.append({
            "x": x,
            "w1": round_fp32r(tile_w1(W1[e])),
            "w2": round_fp32r(tile_w2(W2[e])),
            "b1v": np.ascontiguousarray(np.asarray(b1[e], np.float32)),
            "b2v": np.ascontiguousarray(np.asarray(b2[e], np.float32)),
            "wr": Wr_,
            "brv": br_,
            "esel": onehot,
        })
    return in_maps


def kernel(input_emb, W1, b1, W2, b2, Wr, br):
    global _NC
    if _NC is None:
        _NC = build_kernel()

    in_maps = make_in_maps(input_emb, W1, b1, W2, b2, Wr, br)
    r = run_bass_kernel_spmd(_NC, in_maps, core_ids=list(range(NCORES)))
    outT = np.concatenate([r.results[i]["y"] for i in range(NCORES)], axis=0)
    return np.ascontiguousarray(outT.T).reshape(B, S, D)
